# revision 35
# baseline (speedup 1.0000x reference)
"""ListMLE loss kernel for Trainium2 (8 NeuronCores, data-parallel over batch).

Math (per batch row, N items):
    ss        = scores sorted by `rankings` (gather)
    e         = exp(ss)
    rev[i]    = sum_{j>=i} e[j]            (reverse cumsum)
    loss_row  = sum_{i=0}^{N-2} [ log(rev[i] + eps) - ss[i] ]
    out       = mean(loss_row)

Reformulation used here: with ssr = reverse(ss) and fcs = forward inclusive
cumsum of exp(ssr), rev[i] = fcs[N-1-i] and the loss telescopes to

    loss_row = sum_{k=0}^{N-1} [ log(fcs[k]) - ssr[k] ]

(the k=0 term log(e_r[0]) - ssr[0] is identically zero, which absorbs the
reference's "exclude last position" edge case; eps=1e-10 is negligible since
fcs >= exp(min score) >> eps).

Device-side strategy per core (2048 rows as 16 chunks of 128 partitions,
row-major along the free axis, bf16 payload):
  ACT   : e = Exp(ssr)                       one pass, the only exp engine
  GPSIMD: fcs = per-chunk inclusive cumsum   (tensor_tensor_scan, fp32 state;
          runs at full rate on Pool and keeps DVE free for the folds)
  DVE   : 3 bf16 "fold" passes multiply groups of 8 fcs values -> 128
          group products per chunk, shrinking the Ln pass 8x. Products of 8
          reach ~1e27 — beyond the scalar engine's Ln domain [-2^64, 2^64] —
          so the Ln uses scale=2^-48 (the domain check applies post-scale)
          and the exact +48*ln2 per group is added back on the host.
  ACT   : Ln over group products (3 pieces, placed so their folds are
          already done), accum_out -> per-core log-sums
  PE    : sum(ssr) via ones-stationary matmuls accumulated in PSUM (the
          global sum is all that's needed; the tensor engine is otherwise
          idle and contracts over partitions natively)
  A manually pinned activation table (natural_log_exp_and_others) serves
  both Exp and Ln, avoiding the 1283ns table reload on every Exp<->Ln switch.

The gather + reversal + layout happen host-side while sharding: TRN2 has no
per-partition-indexed gather primitive (GPSIMD indirect ops share indices
across each 16-partition group; DMA gathers are row-granular), so a device
gather would need 16x replicated GPSIMD passes or per-element DMA
descriptors, both orders of magnitude off the memory roofline.
"""

import math
import sys

if "/opt/trn_rl_repo" not in sys.path:
    sys.path.insert(0, "/opt/trn_rl_repo")

from contextlib import ExitStack

import numpy as np

B, N = 16384, 1024
N_CORES = 8
ROWS_PER_CORE = B // N_CORES
P = 128
G = 8  # fcs values per log-group
LN_SCALE = 2.0**-48  # activation scale bringing group products into Ln domain

_CACHE = {}


def _slab_sizes(chunks):
    """Chunk counts per pipelined slab: small slabs at both ends shorten the
    DMA ramp-in and the post-exp scan/fold/ln tail."""
    if chunks <= 4:
        return [1] * chunks
    body = [2] * ((chunks - 4) // 2)
    rem = chunks - 4 - 2 * len(body)
    return [1, 1] + body + ([rem] if rem else []) + [1, 1]


def _combined_table_idx(nc):
    """Index of the activation table containing both Exp and Ln."""
    try:
        from concourse import hw_specs

        names = list(hw_specs.get_activation_tables(nc.m.arch).keys())
        return names.index("natural_log_exp_and_others")
    except Exception:
        return 6  # insertion order in act_info.json as of neuronxcc in image


def build_program(chunks_per_core=ROWS_PER_CORE // P):
    """Build + compile the per-core Bass program (SPMD across 8 cores)."""
    import concourse.bass as bass  # noqa: F401
    import concourse.tile as tile
    from concourse import bacc, mybir

    f32 = mybir.dt.float32
    bf16 = mybir.dt.bfloat16
    Act = mybir.ActivationFunctionType
    Alu = mybir.AluOpType

    sizes = _slab_sizes(chunks_per_core)
    n_slabs = len(sizes)
    starts = [sum(sizes[:i]) for i in range(n_slabs)]

    # Ln pieces (chunk_lo, chunk_hi), emitted after the slab loop: ACT is
    # exp-saturated until then, and each piece is gated by its folds anyway.
    if chunks_per_core == 16:
        ln_pieces = [(0, 8), (8, 14), (14, 16)]
    else:
        ln_pieces = [(0, chunks_per_core)]
    # all fold chains run on gpsimd: the cost model charges Pool TensorTensor
    # at full rate (~747ns/chunk chain, v8 trace) and Pool is otherwise idle,
    # leaving DVE a pure scan stream — its 16 forced scans are the wall
    dve_fold_from = chunks_per_core
    assert len(ln_pieces) <= 4

    nc = bacc.Bacc(
        "TRN2",
        target_bir_lowering=False,
        debug=False,
        enable_asserts=True,
        num_devices=N_CORES,
    )
    ss_d = nc.dram_tensor(
        "ss", [P, chunks_per_core * N], bf16, kind="ExternalInput"
    ).ap()
    log_d = nc.dram_tensor("logsum", [P, 4], f32, kind="ExternalOutput").ap()
    ssq_d = nc.dram_tensor("ssq", [1, 512], f32, kind="ExternalOutput").ap()

    with tile.TileContext(nc) as tc:
        with ExitStack() as ctx:
            pool = ctx.enter_context(tc.tile_pool(name="work", bufs=4))
            spool = ctx.enter_context(tc.tile_pool(name="small", bufs=1))
            ppool = ctx.enter_context(tc.tile_pool(name="psum", bufs=1, space="PSUM"))

            ones = spool.tile([P, 1], bf16)
            nc.gpsimd.memset(ones[:], 1.0)

            flog = spool.tile([P, chunks_per_core, N // G], bf16)
            logd = spool.tile([P, chunks_per_core, N // G], bf16)
            lsum = spool.tile([P, 4], f32)
            nc.gpsimd.memset(lsum[:], 0.0)
            ssacc = ppool.tile([1, 512], f32)

            # pin the Exp+Ln table once; runs during the first DMA
            nc.scalar.add_instruction(
                mybir.InstLoadActFuncSet(
                    name=nc.get_next_instruction_name(),
                    ins=[],
                    outs=[],
                    act_func_set_id=_combined_table_idx(nc),
                )
            )

            n_mm = chunks_per_core * N // 512
            mm_i = 0

            def emit_folds(fcs, c0, cps):
                # per-chunk 3-pass fold chain, on gpsimd (TensorTensor is
                # ISA-legal there, unlike scan) for early chunks and DVE for
                # the tail; bf16 gets 2x on DVE only
                f1 = pool.tile([P, cps, N // 2], bf16, tag=f"f1{cps}")
                f2 = pool.tile([P, cps, N // 4], bf16, tag=f"f2{cps}")
                for c in range(cps):
                    eng = nc.vector if c0 + c >= dve_fold_from else nc.gpsimd
                    eng.tensor_tensor(
                        f1[:, c, :],
                        fcs[:, c, 0 : N // 2],
                        fcs[:, c, N // 2 : N],
                        Alu.mult,
                    )
                    eng.tensor_tensor(
                        f2[:, c, :],
                        f1[:, c, 0 : N // 4],
                        f1[:, c, N // 4 : N // 2],
                        Alu.mult,
                    )
                    eng.tensor_tensor(
                        flog[:, c0 + c, :],
                        f2[:, c, 0 : N // 8],
                        f2[:, c, N // 8 : N // 4],
                        Alu.mult,
                    )

            # folds lag one slab behind the scans in emission order so the
            # fold engines never head-block the next slab's scans
            pending_folds = None
            for s in range(n_slabs):
                cps = sizes[s]
                c0 = starts[s]
                slab_cols = cps * N
                sst = pool.tile([P, cps, N], bf16, tag=f"ss{cps}")
                nc.sync.dma_start(
                    sst[:], ss_d[:, c0 * N : c0 * N + slab_cols]
                )

                # PE: global sum(ssr) — 512-col windows, one long accum group
                for c in range(cps):
                    for w in range(N // 512):
                        nc.tensor.matmul(
                            ssacc[:],
                            ones[:],
                            sst[:, c, w * 512 : (w + 1) * 512],
                            start=(mm_i == 0),
                            stop=(mm_i == n_mm - 1),
                        )
                        mm_i += 1

                es = pool.tile([P, cps, N], bf16, tag=f"es{cps}")
                nc.scalar.activation(es[:], sst[:], Act.Exp)

                fcs = pool.tile([P, cps, N], bf16, tag=f"fcs{cps}")
                for c in range(cps):
                    # scans must run on DVE: walrus's ISA check rejects
                    # TensorTensorScanArith on the Pool engine (CoreSim's cost
                    # model accepts it, hardware codegen does not)
                    nc.vector.tensor_tensor_scan(
                        fcs[:, c, :],
                        es[:, c, :],
                        es[:, c, :],
                        0.0,
                        Alu.add,
                        Alu.bypass,
                    )

                if pending_folds is not None:
                    emit_folds(*pending_folds)
                pending_folds = (fcs, c0, cps)

            if pending_folds is not None:
                emit_folds(*pending_folds)

            # PSUM -> SBUF readout on ACT: it fills ACT's natural idle gap
            # between the last exp and the first (fold-gated) Ln piece, and
            # keeps the copy off DVE's saturated stream
            ssq_s = spool.tile([1, 512], f32)
            nc.scalar.copy(ssq_s[:], ssacc[:])
            nc.sync.dma_start(ssq_d[:], ssq_s[:])

            for piece_i, (lo, hi) in enumerate(ln_pieces):
                nc.scalar.activation(
                    logd[:, lo:hi, :],
                    flog[:, lo:hi, :],
                    Act.Ln,
                    scale=LN_SCALE,
                    accum_out=lsum[:, piece_i : piece_i + 1],
                )
                # ship each piece as soon as it lands
                nc.sync.dma_start(
                    log_d[:, piece_i : piece_i + 1], lsum[:, piece_i : piece_i + 1]
                )

    nc.compile()
    return nc, len(ln_pieces)


def _get_program(chunks_per_core=ROWS_PER_CORE // P):
    """Returns (nc, n_ln_pieces)."""
    if chunks_per_core not in _CACHE:
        _CACHE[chunks_per_core] = build_program(chunks_per_core)
    return _CACHE[chunks_per_core]


def prep_inputs(scores: np.ndarray, rankings: np.ndarray):
    """Host prep: gather, reverse, chunk layout, bf16. Returns per-core
    in_maps for run_bass_kernel_spmd."""
    import ml_dtypes

    scores = np.asarray(scores, dtype=np.float32)
    rankings = np.asarray(rankings)
    rows = scores.shape[0]
    ss = np.take_along_axis(scores, rankings, axis=1)
    ssr = ss[:, ::-1].astype(ml_dtypes.bfloat16)
    rpc = rows // N_CORES
    cpc = rpc // P
    in_maps = []
    for c in range(N_CORES):
        block = ssr[c * rpc : (c + 1) * rpc]  # [rpc, N]
        # chunk k, partition p <- row k*P + p; free axis = chunk-major
        lay = np.ascontiguousarray(
            block.reshape(cpc, P, N).transpose(1, 0, 2).reshape(P, cpc * N)
        )
        in_maps.append({"ss": lay})
    return in_maps


def unscale_correction(rows):
    """Exact Ln correction for the scaled fold2: +48*ln2 per group."""
    groups = rows * (N // G)
    return groups * 48.0 * math.log(2.0)


def kernel(scores: np.ndarray, rankings: np.ndarray) -> np.ndarray:
    from concourse import bass_utils

    scores = np.asarray(scores, dtype=np.float32)
    rankings = np.asarray(rankings)
    assert scores.shape == (B, N) and rankings.shape == (B, N)

    in_maps = prep_inputs(scores, rankings)
    nc, n_pieces = _get_program()
    res = bass_utils.run_bass_kernel_spmd(nc, in_maps, core_ids=list(range(N_CORES)))
    total = unscale_correction(B)
    for r in res.results:
        total += float(r["logsum"][:, :n_pieces].astype(np.float64).sum())
        total -= float(r["ssq"].astype(np.float64).sum())
    return np.float32(total / B)
